# revision 28
# baseline (speedup 1.0000x reference)
"""Trainium2 Bass kernel for nn_Block_CD (dual-stream patch-embed + attention).

Math per stream (x / y), tokens = (sample, l), l = 25 positions:
  xpc = centered conv3x3(img) + pos + conv_b  (im2col-ext matmul with the
                                               channel-mean folded out, so
                                               E[xpc]=0 and var=E[xpc^2])
  xln = xpc * rs,  rs = exp(-0.5*ln(256*var + 256*eps))   (ln+exp: one ACT
                                               table set shared with the
                                               softmax exp -> no set switches)
  qkv = (16 * qkv_w * g).T @ xln              (feature-major, 128-row halves)
  attention per sample via 32x32 tensor-engine array packing
  (tile_position=(32h,32h)): scores = k_s^T q_s per head, E = exp(SCALE*sc),
  den via an all-ones [25,32] stationary, av = vts_s^T E, avn = av/den
  out = proj^T @ avn + W_res^T @ ic           (residual conv recomputed into
                                               the proj psum; W_res ext rows
                                               carry conv_b + proj_b, no pos;
                                               DMA'd to DRAM straight from
                                               PSUM)
Output device layout: [512, B_loc*25] f32, rearranged on host.
Sharding: pure data parallel, B=8192 over 8 cores.
The per-stream chunk loop is a hardware For_i loop (staggered reset); every
stage is sliced per 16-sample psum slice so the engines pipeline.
"""
import sys
sys.path.insert(0, "/opt/trn_rl_repo")
import numpy as np
import ml_dtypes

import concourse.bass as bass
import concourse.mybir as mybir
import concourse.tile as tile
from concourse import bacc, bass_utils
from concourse.bass import ds

bf16 = mybir.dt.bfloat16
f32 = mybir.dt.float32
AF = mybir.ActivationFunctionType
ALU = mybir.AluOpType

DIM = 256
HEADS = 8
HD = 32
L = 25
SCALE = HD ** -0.5
LN_EPS = 1e-5
NCORES = 8
B = 8192
B_LOC = B // NCORES

S_O = 64          # samples per outer chunk
S_I = 16          # samples per inner psum slice
N_I = S_I * L     # 400
TOK_O = S_O * L   # 1600

_CACHE = {}


def _to_bf16(a):
    return np.asarray(a, np.float32).astype(ml_dtypes.bfloat16)


def _host_prep(inputs):
    pos = np.asarray(inputs["pos_embed"], np.float32).reshape(L, DIM)
    ln_g = np.asarray(inputs["ln_g"], np.float32)
    ln_b = np.asarray(inputs["ln_b"], np.float32)

    def im2col_ext(img):
        p = np.pad(np.asarray(img, np.float32), ((0, 0), (0, 0), (1, 1), (1, 1)))
        Bn = img.shape[0]
        cols = np.empty((Bn, L, 52), np.float32)
        idx = 0
        for c in range(3):
            for di in range(3):
                for dj in range(3):
                    cols[:, :, idx] = p[:, c, di:di + 5, dj:dj + 5].reshape(Bn, L)
                    idx += 1
        cols[:, :, 27:] = np.eye(L, dtype=np.float32)[None]
        return cols  # [B, 25, 52]

    prep = {}
    for nm, ik, cw, cb, qw, pw, pb in (
        ("x", "x", "conv1_w", "conv1_b", "qkv_x_w", "proj_x_w", "proj_x_b"),
        ("y", "y", "conv2_w", "conv2_b", "qkv_y_w", "proj_y_w", "proj_y_b"),
    ):
        conv_w = np.asarray(inputs[cw], np.float32)
        conv_b = np.asarray(inputs[cb], np.float32)
        qkv_w = np.asarray(inputs[qw], np.float32)
        proj_w = np.asarray(inputs[pw], np.float32)
        proj_b = np.asarray(inputs[pb], np.float32)

        w_emb = np.empty((52, DIM), np.float32)
        w_emb[:27] = conv_w.reshape(DIM, 27).T
        w_emb[27:] = pos + conv_b[None, :]
        # fold LN mean-centering into the embed weights: mu is linear in ic
        # (mu = mean_c(w_emb)^T ic), so the centered embed comes out of the
        # matmul directly and E[xpc^2] is the variance
        prep[f"wemb_{nm}"] = w_emb - w_emb.mean(axis=1, keepdims=True)

        # residual + biases recomputed into the proj psum: conv_out + conv_b
        # + proj_b (no pos)
        w_res = np.empty((52, DIM), np.float32)
        w_res[:27] = conv_w.reshape(DIM, 27).T
        w_res[27:] = (conv_b + proj_b)[None, :]
        prep[f"wres_{nm}"] = w_res

        wq = (16.0 * qkv_w * ln_g[None, :]).T          # [256, 768]
        prep[f"wqkv_{nm}"] = np.concatenate([wq[0:128], wq[128:256]], axis=1)  # [128,1536]
        c = qkv_w @ ln_b
        assert np.abs(c).max() < 1e-6, "nonzero ln_b fold not supported"

        wp = proj_w.T                                   # [256, 256] lhsT
        prep[f"wproj_{nm}"] = np.concatenate([wp[0:128], wp[128:256]], axis=1)  # [128,512]

        prep[f"ic_{nm}"] = im2col_ext(inputs[ik])

    prep["ones_ln"] = np.full((128, 128), 1.0 / DIM, np.float32)
    # per-32-block [25->1, 7->0] column stationary for the softmax denominator
    # (each 32-row quadrant needs its own copy for tile_position loads)
    o25 = np.zeros((128, 32), np.float32)
    for h in range(4):
        o25[32 * h:32 * h + L, :] = 1.0
    prep["ones25"] = o25
    return prep


def _build_kernel(nc, tc, b_loc, n_chunk=None, repeat=1):
    import contextlib
    ctx = contextlib.ExitStack()
    if n_chunk is None:
        n_chunk = b_loc // S_O
    n_sl = S_O // S_I

    dram = {}
    for nm in ("x", "y"):
        dram[f"ic_{nm}"] = nc.dram_tensor(f"ic_{nm}", [52, b_loc * L], bf16, kind="ExternalInput").ap()
        dram[f"wemb_{nm}"] = nc.dram_tensor(f"wemb_{nm}", [52, DIM], bf16, kind="ExternalInput").ap()
        dram[f"wres_{nm}"] = nc.dram_tensor(f"wres_{nm}", [52, DIM], bf16, kind="ExternalInput").ap()
        dram[f"wqkv_{nm}"] = nc.dram_tensor(f"wqkv_{nm}", [128, 1536], bf16, kind="ExternalInput").ap()
        dram[f"wproj_{nm}"] = nc.dram_tensor(f"wproj_{nm}", [128, 512], bf16, kind="ExternalInput").ap()
    dram["ones_ln"] = nc.dram_tensor("ones_ln", [128, 128], bf16, kind="ExternalInput").ap()
    dram["ones25"] = nc.dram_tensor("ones25", [128, 32], bf16, kind="ExternalInput").ap()
    out_d = nc.dram_tensor("out", [2 * DIM, b_loc * L], bf16, kind="ExternalOutput").ap()

    const = ctx.enter_context(tc.tile_pool(name="const", bufs=1))
    sb = ctx.enter_context(tc.tile_pool(name="sb", bufs=1))
    ps = ctx.enter_context(tc.tile_pool(name="ps", bufs=2, space="PSUM"))

    W = {}
    for nm in ("x", "y"):
        for key, shp in (("emb", [52, DIM]), ("res", [52, DIM]),
                         ("qkv", [128, 1536]), ("proj", [128, 512])):
            W[f"{key}_{nm}"] = const.tile(shp, bf16, tag=f"{key}{nm}", name=f"{key}{nm}")
            nc.sync.dma_start(W[f"{key}_{nm}"][:, :], dram[f"w{key}_{nm}"])
    W["ones_ln"] = const.tile([128, 128], bf16, tag="ones_ln", name="ones_ln")
    nc.sync.dma_start(W["ones_ln"][:, :], dram["ones_ln"])
    W["ones25"] = const.tile([128, 32], bf16, tag="ones25", name="ones25")
    nc.sync.dma_start(W["ones25"][:, :], dram["ones25"])
    eps256 = const.tile([128, 1], f32, tag="eps256", name="eps256")
    nc.vector.memset(eps256[:, :], 256.0 * LN_EPS)

    def make_stream(nm):
        T = {"nm": nm}
        T["ic"] = sb.tile([52, TOK_O], bf16, tag=f"ic_{nm}", name=f"ic_{nm}", bufs=2)
        T["xp"] = [sb.tile([128, TOK_O], bf16, tag=f"xp{t}_{nm}", name=f"xp{t}_{nm}", bufs=2) for t in range(2)]
        T["sq"] = [sb.tile([128, TOK_O], bf16, tag=f"sq{t}_{nm}", name=f"sq{t}_{nm}", bufs=2) for t in range(2)]
        T["rs"] = sb.tile([128, TOK_O], bf16, tag=f"rs_{nm}", name=f"rs_{nm}")
        T["xln"] = [sb.tile([128, TOK_O], bf16, tag=f"xln{t}_{nm}", name=f"xln{t}_{nm}", bufs=2) for t in range(2)]
        T["qkv"] = [sb.tile([128, TOK_O], bf16, tag=f"qkv{m}_{nm}", name=f"qkv{m}_{nm}", bufs=2) for m in range(2)]
        T["ebuf"] = [sb.tile([128, L * S_O], bf16, tag=f"e{g}_{nm}", name=f"e{g}_{nm}", bufs=2) for g in range(2)]
        T["avn"] = [sb.tile([128, L * S_O], bf16, tag=f"avn{g}_{nm}", name=f"avn{g}_{nm}") for g in range(2)]
        T["ksl"] = {}
        T["vts"] = {}
        return T

    def phase_a(T, s, tok0):
        nm = T["nm"]
        sl = slice(s * N_I, (s + 1) * N_I)
        if s == 0:
            nc.sync.dma_start(T["ic"][:, :], dram[f"ic_{nm}"][:, ds(tok0, TOK_O)])
        ic, xp, sq, rs, xln, qkv = (T["ic"], T["xp"], T["sq"], T["rs"], T["xln"], T["qkv"])
        for t in range(2):
            pt = ps.tile([128, N_I], f32, tag="mm", bufs=4, name="pt")
            nc.tensor.matmul(pt[:, :], W[f"emb_{nm}"][:, 128 * t:128 * (t + 1)],
                             ic[:, sl], start=True, stop=True)
            if t == 0:
                nc.scalar.activation(xp[t][:, sl], pt[:, :], AF.Copy)
            else:
                nc.vector.tensor_copy(xp[t][:, sl], pt[:, :])
            nc.gpsimd.tensor_mul(sq[t][:, sl], xp[t][:, sl], xp[t][:, sl])
        pv = ps.tile([128, N_I], f32, tag="mm", bufs=4, name="pv")
        nc.tensor.matmul(pv[:, :], W["ones_ln"][:, :], sq[0][:, sl], start=True, stop=False)
        nc.tensor.matmul(pv[:, :], W["ones_ln"][:, :], sq[1][:, sl], start=False, stop=True)
        # rs = 1/(16*sqrt(var+eps)) = exp(-0.5*ln(256*var + 256*eps))
        t2 = sb.tile([128, N_I], f32, tag="t2", bufs=2, name="t2")
        nc.scalar.activation(t2[:, :], pv[:, :], AF.Ln, bias=eps256[:, 0:1], scale=256.0)
        nc.scalar.activation(rs[:, sl], t2[:, :], AF.Exp, scale=-0.5)
        for t in range(2):
            nc.gpsimd.tensor_mul(xln[t][:, sl], xp[t][:, sl], rs[:, sl])
        # q for head groups 0/1: [128=(h,d), (j,l)]
        for g in range(2):
            pq = ps.tile([128, N_I], f32, tag="mm", bufs=4, name="pq")
            nc.tensor.matmul(pq[:, :], W[f"qkv_{nm}"][:, 128 * g:128 * (g + 1)],
                             xln[0][:, sl], start=True, stop=False)
            nc.tensor.matmul(pq[:, :], W[f"qkv_{nm}"][:, 768 + 128 * g:768 + 128 * (g + 1)],
                             xln[1][:, sl], start=False, stop=True)
            nc.vector.tensor_copy(qkv[g][:, sl], pq[:, :])
        # k, plain [128=(h,d), (j,l)] layout (scores stationaries slice it)
        for g in range(2):
            pq = ps.tile([128, N_I], f32, tag="mm", bufs=4, name="pk")
            nc.tensor.matmul(pq[:, :], W[f"qkv_{nm}"][:, 128 * (2 + g):128 * (3 + g)],
                             xln[0][:, sl], start=True, stop=False)
            nc.tensor.matmul(pq[:, :], W[f"qkv_{nm}"][:, 768 + 128 * (2 + g):768 + 128 * (3 + g)],
                             xln[1][:, sl], start=False, stop=True)
            T["ksl"][(g, s)] = sb.tile([128, N_I], bf16, tag=f"ksl{g}_{nm}",
                                       name=f"ksl{g}_{nm}", bufs=n_sl)
            if g == 0:
                nc.scalar.activation(T["ksl"][(g, s)][:, :], pq[:, :], AF.Copy)
            else:
                nc.vector.tensor_copy(T["ksl"][(g, s)][:, :], pq[:, :])
        # v -> 32-padded staging -> per-32-block transpose [(h,l), (j,d)]
        for g in range(2):
            pq = ps.tile([128, N_I], f32, tag="mm", bufs=4, name="pvv")
            nc.tensor.matmul(pq[:, :], W[f"qkv_{nm}"][:, 128 * (4 + g):128 * (5 + g)],
                             xln[0][:, sl], start=True, stop=False)
            nc.tensor.matmul(pq[:, :], W[f"qkv_{nm}"][:, 768 + 128 * (4 + g):768 + 128 * (5 + g)],
                             xln[1][:, sl], start=False, stop=True)
            vsl = sb.tile([128, 32 * S_I], bf16, tag="vsl", name="vsl", bufs=2)
            nc.gpsimd.memset(
                vsl[:, :].rearrange("p (j l) -> p j l", l=32)[:, :, L:32], 0.0)
            nc.scalar.activation(
                vsl[:, :].rearrange("p (j l) -> p j l", l=32)[:, :, 0:L],
                pq[:, :].rearrange("p (j l) -> p j l", l=L), AF.Copy)
            T["vts"][(g, s)] = sb.tile([128, 32 * S_I], bf16, tag=f"vts{g}_{nm}",
                                       name=f"vts{g}_{nm}", bufs=n_sl)
            nc.vector.transpose(T["vts"][(g, s)][:, :], vsl[:, :])

    def scores(T, g, s):
        sl = slice(s * N_I, (s + 1) * N_I)
        sc = ps.tile([128, N_I], f32, tag="sc", name="sc", bufs=3)
        for j in range(S_I):
            for h in range(4):
                nc.tensor.matmul(
                    sc[32 * h:32 * h + L, j * L:(j + 1) * L],
                    T["ksl"][(g, s)][32 * h:32 * h + 32, j * L:(j + 1) * L],
                    T["qkv"][g][32 * h:32 * h + 32, (s * S_I + j) * L:(s * S_I + j + 1) * L],
                    start=True, stop=True, tile_position=(32 * h, 32 * h))
        nc.scalar.activation(T["ebuf"][g][:, sl], sc[:, :], AF.Exp, scale=SCALE)

    def av_norm(T, g, s):
        sl = slice(s * N_I, (s + 1) * N_I)
        ebuf = T["ebuf"]
        dn = ps.tile([128, N_I], f32, tag="mm", bufs=4, name="dn")
        for h in range(4):
            nc.tensor.matmul(dn[32 * h:32 * h + 32, :], W["ones25"][32 * h:32 * h + L, :],
                             ebuf[g][32 * h:32 * h + L, sl],
                             start=True, stop=True, tile_position=(32 * h, 32 * h))
        rden = sb.tile([128, N_I], f32, tag="rden", name="rden", bufs=2)
        nc.vector.reciprocal_approx_fast(rden[:, :], dn[:, :])
        av = ps.tile([128, N_I], f32, tag="av", name="av", bufs=1)
        for j in range(S_I):
            for h in range(4):
                nc.tensor.matmul(
                    av[32 * h:32 * h + 32, j * L:(j + 1) * L],
                    T["vts"][(g, s)][32 * h:32 * h + L, 32 * j:32 * (j + 1)],
                    ebuf[g][32 * h:32 * h + L, (s * S_I + j) * L:(s * S_I + j + 1) * L],
                    start=True, stop=True, tile_position=(32 * h, 32 * h))
        nc.vector.tensor_mul(T["avn"][g][:, sl], av[:, :], rden[:, :])

    def proj(T, s, tok0):
        nm = T["nm"]
        sl = slice(s * N_I, (s + 1) * N_I)
        ob = 0 if nm == "x" else DIM
        for t in range(2):
            pp = ps.tile([128, N_I], f32, tag="mm", bufs=4, name="pp")
            nc.tensor.matmul(pp[:, :], W[f"proj_{nm}"][:, 128 * t:128 * (t + 1)],
                             T["avn"][0][:, sl], start=True, stop=False)
            nc.tensor.matmul(pp[:, :], W[f"proj_{nm}"][:, 256 + 128 * t:256 + 128 * (t + 1)],
                             T["avn"][1][:, sl], start=False, stop=False)
            nc.tensor.matmul(pp[:, :], W[f"res_{nm}"][:, 128 * t:128 * (t + 1)],
                             T["ic"][:, sl], start=False, stop=True)
            o2 = sb.tile([128, N_I], bf16, tag="o2", name="o2", bufs=2)
            if t == 0:
                nc.scalar.activation(o2[:, :], pp[:, :], AF.Copy)
            else:
                nc.vector.tensor_copy(o2[:, :], pp[:, :])
            nc.sync.dma_start(
                out_d[ob + 128 * t: ob + 128 * (t + 1), ds(tok0 + s * N_I, N_I)],
                o2[:, :])

    def body(tok0):
        # two independent stream pipelines (x and y), interleaved step by
        # step so each engine always has a second chain to work on while the
        # other chain's cross-engine dependency drains
        TX, TY = make_stream("x"), make_stream("y")
        both = (TX, TY)
        steps = [(g, s) for s in range(n_sl) for g in range(2)]
        for T in both:
            phase_a(T, 0, tok0)
        for T in both:
            phase_a(T, 1, tok0)
        for T in both:
            scores(T, *steps[0])
        for i, (g, s) in enumerate(steps):
            if g == 0 and s + 2 < n_sl:
                for T in both:
                    phase_a(T, s + 2, tok0)
            if i + 1 < len(steps):
                for T in both:
                    scores(T, *steps[i + 1])
            for T in both:
                av_norm(T, g, s)
            if g == 1:
                for T in both:
                    proj(T, s, tok0)

    def run_all():
        with tc.For_i(0, n_chunk * TOK_O, TOK_O, name="chunk",
                      staggered_reset=True) as tok0:
            body(tok0)

    # repeat>1 is a timing variant: statically duplicate the whole kernel
    # (same data, same outputs) so a bench can take a slope over repeats with
    # identical program structure
    for _ in range(repeat):
        run_all()
    ctx.close()


def _get_nc(b_loc, n_chunk=None, repeat=1):
    key = (b_loc, n_chunk, repeat)
    if key in _CACHE:
        return _CACHE[key]
    nc = bacc.Bacc("TRN2", target_bir_lowering=False, debug=False,
                   enable_asserts=False, num_devices=NCORES)
    with tile.TileContext(nc, trace_sim=False) as tc:
        _build_kernel(nc, tc, b_loc, n_chunk, repeat)
    nc.compile()
    bass.Bass.finalize(nc)
    _CACHE[key] = nc
    return nc


def _in_maps(prep, b_loc, ncores):
    maps = []
    for c in range(ncores):
        s0 = c * b_loc
        m = {}
        for nm in ("x", "y"):
            ic = prep[f"ic_{nm}"][s0:s0 + b_loc].reshape(b_loc * L, 52).T
            m[f"ic_{nm}"] = _to_bf16(np.ascontiguousarray(ic))
            m[f"wemb_{nm}"] = _to_bf16(prep[f"wemb_{nm}"])
            m[f"wres_{nm}"] = _to_bf16(prep[f"wres_{nm}"])
            m[f"wqkv_{nm}"] = _to_bf16(prep[f"wqkv_{nm}"])
            m[f"wproj_{nm}"] = _to_bf16(prep[f"wproj_{nm}"])
        m["ones_ln"] = _to_bf16(prep["ones_ln"])
        m["ones25"] = _to_bf16(prep["ones25"])
        maps.append(m)
    return maps


def kernel(**inputs):
    prep = _host_prep(inputs)
    nc = _get_nc(B_LOC)
    res = bass_utils.run_bass_kernel_spmd(nc, _in_maps(prep, B_LOC, NCORES),
                                          core_ids=list(range(NCORES)))
    outs = [res.results[c]["out"] for c in range(NCORES)]
    full = np.concatenate(
        [np.asarray(o, np.float32).reshape(2 * DIM, B_LOC, L).transpose(1, 0, 2)
         for o in outs], axis=0)
    return np.ascontiguousarray(full.reshape(B, 2 * DIM, 5, 5))
